# revision 5
# baseline (speedup 1.0000x reference)
"""GRU (equinox GRUCell scan) Trainium2 Bass kernel.

Problem: x (T=4096, B=32, D=256), weights W_ih (768,256), W_hh (768,256),
b (768,), b_n (256,), initial_state (32, 256) -> h_sequence (T, B, H=256).

Strategy: data-parallel over batch across 8 cores (4 batch rows per core).
Per core:
  Phase A: xg = x @ W_ih.T + b for all T, computed in gate-major layout
           (gate-chunk on partitions) and staged to DRAM.
  Phase B: sequential recurrence; per step small matmuls accumulate
           h @ W_hh.T (+ b_n via a ones-row matmul) into PSUM, sigmoid/tanh
           on the scalar engine, gate arithmetic on the vector engine.
"""

import numpy as np
from contextlib import ExitStack

import concourse.bass as bass
import concourse.bacc as bacc
import concourse.tile as tile
from concourse import mybir
from concourse import bass_utils
from concourse.bass import ds, ts
from concourse.masks import make_identity

T, B, D, H = 4096, 32, 256, 256
NCORES = 8
BC = B // NCORES          # batch per core = 4
G3 = 3 * H                # 768
GC = G3 // 128            # 6 gate chunks: r=0..1, z=2..3, n=4..5
KC = H // 128             # 2 contraction chunks
DC = D // 128             # 2 input-dim chunks
F32 = mybir.dt.float32

TBA = 128                 # phase A steps per block (512 tokens)
NBA = T // TBA            # 32
TB = 256                  # phase B steps per block
NB = T // TB              # 16
UNROLL = 16

AF = mybir.ActivationFunctionType


def _build_gru(tc: tile.TileContext, aps: dict):
    nc = tc.nc
    x = aps["x"]                  # (T, BC, D)
    h0 = aps["initial_state"]     # (BC, H)
    W_ih = aps["W_ih"]            # (G3, D)
    W_hh = aps["W_hh"]            # (G3, H)
    b_ = aps["b"]                 # (G3,)
    b_n = aps["b_n"]              # (H,)
    y = aps["y"]                  # (T, BC, H)
    xg_stage = aps["xg_stage"]    # (GC, 128, T*BC)

    # gate-major views of DRAM tensors
    xg_r = xg_stage.rearrange("c p tb -> p c tb")              # (128, GC, T*BC)
    y_r = y.rearrange("t b (k p) -> p k (t b)", p=128)         # (128, KC, T*BC)
    h0_r = h0.rearrange("b (k p) -> p k b", p=128)             # (128, KC, BC)

    with ExitStack() as octx:
        singles = octx.enter_context(tc.tile_pool(name="singles", bufs=1))

        # Weights in lhsT form: W*_sb[p, k, g] = W[g, k*128+p]
        Wih_sb = singles.tile([128, DC, G3], F32)
        Wih_r = W_ih.rearrange("g (k p) -> p k g", p=128)
        for k in range(DC):
            nc.sync.dma_start(Wih_sb[:, k, :], Wih_r[:, k, :])
        Whh_sb = singles.tile([128, KC, G3], F32)
        Whh_r = W_hh.rearrange("g (k p) -> p k g", p=128)
        for k in range(KC):
            nc.sync.dma_start(Whh_sb[:, k, :], Whh_r[:, k, :])
        b_sb = singles.tile([1, G3], F32)
        nc.sync.dma_start(b_sb, b_.rearrange("(o g) -> o g", o=1))
        bn_sb = singles.tile([1, H], F32)
        nc.sync.dma_start(bn_sb, b_n.rearrange("(o g) -> o g", o=1))
        ones_bc = singles.tile([1, BC], F32)
        nc.vector.memset(ones_bc, 1.0)
        onesA = singles.tile([1, TBA * BC], F32)
        nc.vector.memset(onesA, 1.0)
        ident = singles.tile([128, 128], F32)
        make_identity(nc, ident)

        # ---------------- Phase A: xg = x @ W_ih.T + b ----------------
        with ExitStack() as actx:
            a_in = actx.enter_context(tc.tile_pool(name="a_in", bufs=2))
            a_xt = actx.enter_context(tc.tile_pool(name="a_xt", bufs=2))
            a_out = actx.enter_context(tc.tile_pool(name="a_out", bufs=2))
            a_tp = actx.enter_context(
                tc.tile_pool(name="a_tp", bufs=2, space="PSUM"))
            a_ps = actx.enter_context(
                tc.tile_pool(name="a_ps", bufs=2, space="PSUM"))

            NTOK = TBA * BC  # 512 tokens per block
            for blk in range(NBA):
                xin = a_in.tile([128, 4, DC, 128], F32)  # [tokp, grp, kd, d]
                for g in range(4):
                    t0 = blk * TBA + g * (TBA // 4)
                    nc.sync.dma_start(
                        xin[:, g],
                        x[t0:t0 + TBA // 4].rearrange(
                            "t b (k d) -> (t b) k d", d=128),
                    )
                xT = a_xt.tile([128, DC, NTOK], F32)     # [d, kd, tok]
                for g in range(4):
                    for kd in range(DC):
                        tp = a_tp.tile([128, 128], F32)
                        nc.tensor.transpose(tp, xin[:, g, kd], ident)
                        nc.scalar.copy(xT[:, kd, ts(g, 128)], tp)
                xga = a_out.tile([128, GC, NTOK], F32)
                for c in range(GC):
                    ps = a_ps.tile([128, NTOK], F32)
                    nc.tensor.matmul(ps, lhsT=b_sb[0:1, ts(c, 128)],
                                     rhs=onesA[0:1, :], start=True, stop=False)
                    for kd in range(DC):
                        nc.tensor.matmul(ps, lhsT=Wih_sb[:, kd, ts(c, 128)],
                                         rhs=xT[:, kd, :],
                                         start=False, stop=(kd == DC - 1))
                    nc.vector.tensor_copy(xga[:, c, :], ps)
                nc.sync.dma_start(
                    xg_r[:, :, ds(blk * NTOK, NTOK)], xga)

        # Phase A writes xg_stage (raw DRAM tensor, not a pool tile) and
        # phase B reads it; force ordering across the DMA queues.
        tc.strict_bb_all_engine_barrier()

        # ---------------- Phase B: recurrence ----------------
        with ExitStack() as bctx:
            xg_pool = bctx.enter_context(tc.tile_pool(name="xg", bufs=2))
            h_pool = bctx.enter_context(tc.tile_pool(name="hh", bufs=2))
            sm = bctx.enter_context(tc.tile_pool(name="sm", bufs=3))
            ps_rz = bctx.enter_context(
                tc.tile_pool(name="ps_rz", bufs=2, space="PSUM"))
            ps_c2 = bctx.enter_context(
                tc.tile_pool(name="ps_c2", bufs=2, space="PSUM"))

            prev_hist = None
            for blk in range(NB):
                xg_sb = xg_pool.tile([128, GC, TB * BC], F32)
                nc.sync.dma_start(
                    xg_sb, xg_r[:, :, ds(blk * TB * BC, TB * BC)])
                h_hist = h_pool.tile([128, KC, (TB + 1) * BC], F32)
                if blk == 0:
                    for k in range(KC):
                        nc.sync.dma_start(h_hist[:, k, 0:BC], h0_r[:, k, :])
                else:
                    nc.vector.tensor_copy(
                        h_hist[:, :, 0:BC],
                        prev_hist[:, :, ds(TB * BC, BC)])

                def step(t):
                    h_prev = h_hist[:, :, ts(t, BC)]
                    rz_ps = ps_rz.tile([128, 4, BC], F32)
                    for c in range(4):
                        for k in range(KC):
                            nc.tensor.matmul(
                                rz_ps[:, c, :],
                                lhsT=Whh_sb[:, k, ts(c, 128)],
                                rhs=h_prev[:, k, :],
                                start=(k == 0), stop=(k == KC - 1))
                    c2_ps = ps_c2.tile([128, 2, BC], F32)
                    for cc in range(2):
                        c = 4 + cc
                        nc.tensor.matmul(
                            c2_ps[:, cc, :],
                            lhsT=bn_sb[0:1, ts(cc, 128)],
                            rhs=ones_bc[0:1, :], start=True, stop=False)
                        for k in range(KC):
                            nc.tensor.matmul(
                                c2_ps[:, cc, :],
                                lhsT=Whh_sb[:, k, ts(c, 128)],
                                rhs=h_prev[:, k, :],
                                start=False, stop=(k == KC - 1))
                    # argrz = xg_rz + h@W_rz.T   (in-place in PSUM)
                    nc.vector.tensor_add(
                        rz_ps, rz_ps, xg_sb[:, 0:4, ts(t, BC)])
                    rz_sb = sm.tile([128, 4, BC], F32, tag="rz")
                    nc.scalar.activation(rz_sb, rz_ps, AF.Sigmoid)
                    # c2 = r * (h@W_n.T + b_n); n_arg = c2 + xg_n
                    nc.vector.tensor_mul(c2_ps, rz_sb[:, 0:2, :], c2_ps)
                    nc.vector.tensor_add(
                        c2_ps, c2_ps, xg_sb[:, 4:6, ts(t, BC)])
                    n_sb = sm.tile([128, 2, BC], F32, tag="n")
                    nc.scalar.activation(n_sb, c2_ps, AF.Tanh)
                    # h' = n + z*(h - n)
                    u_sb = sm.tile([128, 2, BC], F32, tag="u")
                    nc.vector.tensor_sub(u_sb, h_prev, n_sb)
                    nc.vector.tensor_mul(u_sb, rz_sb[:, 2:4, :], u_sb)
                    nc.vector.tensor_add(
                        h_hist[:, :, ts(t + 1, BC)], n_sb, u_sb)

                tc.For_i_unrolled(0, TB, 1, step, max_unroll=UNROLL)

                for k in range(KC):
                    nc.sync.dma_start(
                        y_r[:, k, ds(blk * TB * BC, TB * BC)],
                        h_hist[:, k, BC:(TB + 1) * BC])
                prev_hist = h_hist


_BUILT = None


def _build():
    global _BUILT
    if _BUILT is not None:
        return _BUILT
    nc = bacc.Bacc("TRN2", target_bir_lowering=False, debug=False,
                   num_devices=NCORES)
    aps = {}
    aps["x"] = nc.dram_tensor("x", (T, BC, D), F32, kind="ExternalInput").ap()
    aps["initial_state"] = nc.dram_tensor(
        "initial_state", (BC, H), F32, kind="ExternalInput").ap()
    aps["W_ih"] = nc.dram_tensor("W_ih", (G3, D), F32,
                                 kind="ExternalInput").ap()
    aps["W_hh"] = nc.dram_tensor("W_hh", (G3, H), F32,
                                 kind="ExternalInput").ap()
    aps["b"] = nc.dram_tensor("b", (G3,), F32, kind="ExternalInput").ap()
    aps["b_n"] = nc.dram_tensor("b_n", (H,), F32, kind="ExternalInput").ap()
    aps["y"] = nc.dram_tensor("y", (T, BC, H), F32,
                              kind="ExternalOutput").ap()
    aps["xg_stage"] = nc.dram_tensor("xg_stage", (GC, 128, T * BC), F32,
                                     kind="Internal").ap()
    with tile.TileContext(nc) as tc:
        _build_gru(tc, aps)
    nc.compile()
    _BUILT = nc
    return nc


def run(inputs: dict, trace: bool = False):
    nc = _build()
    in_maps = []
    for i in range(NCORES):
        sl = slice(i * BC, (i + 1) * BC)
        in_maps.append({
            "x": np.ascontiguousarray(
                np.asarray(inputs["x"], dtype=np.float32)[:, sl, :]),
            "initial_state": np.ascontiguousarray(
                np.asarray(inputs["initial_state"], dtype=np.float32)[sl]),
            "W_ih": np.ascontiguousarray(
                np.asarray(inputs["W_ih"], dtype=np.float32)),
            "W_hh": np.ascontiguousarray(
                np.asarray(inputs["W_hh"], dtype=np.float32)),
            "b": np.ascontiguousarray(
                np.asarray(inputs["b"], dtype=np.float32)),
            "b_n": np.ascontiguousarray(
                np.asarray(inputs["b_n"], dtype=np.float32)),
        })
    res = bass_utils.run_bass_kernel_spmd(
        nc, in_maps, core_ids=list(range(NCORES)), trace=trace)
    outs = res.results
    out = np.concatenate([outs[i]["y"] for i in range(NCORES)], axis=1)
    return out.astype(np.float32), res


def kernel(**inputs) -> np.ndarray:
    out, _ = run(inputs, trace=False)
    return out


# revision 11
# speedup vs baseline: 1.2683x; 1.2683x over previous
"""GRU (equinox GRUCell scan) Trainium2 Bass kernel.

Problem: x (T=4096, B=32, D=256), weights W_ih (768,256), W_hh (768,256),
b (768,), b_n (256,), initial_state (32, 256) -> h_sequence (T, B, H=256).

Strategy: data-parallel over batch across 8 cores (4 batch rows per core).
Per core:
  Phase A: xg = x @ W_ih.T + b for all T in fp16, gate-major, staged to DRAM.
  Phase B: sequential recurrence, one dynamic loop over all T with in-loop
           ping-pong DMA. fp16 weights/state for the matmuls (FWL weight
           loads), all per-step access patterns static. xg is accumulated
           into PSUM via identity matmuls so the sigmoid reads PSUM directly.
"""

import numpy as np
from contextlib import ExitStack

import concourse.bass as bass
import concourse.bacc as bacc
import concourse.tile as tile
from concourse import mybir
from concourse import bass_utils
from concourse.bass import ds, ts
from concourse.masks import make_identity

T, B, D, H = 4096, 32, 256, 256
NCORES = 8
BC = B // NCORES          # batch per core = 4
G3 = 3 * H                # 768
GC = G3 // 128            # 6 gate chunks: r=0..1, z=2..3, n=4..5
KC = H // 128             # 2 contraction chunks
DC = D // 128             # 2 input-dim chunks
F32 = mybir.dt.float32
F16 = mybir.dt.float16

TBA = 128                 # phase A steps per block (512 tokens)
NBA = T // TBA            # 32
HB = 16                   # phase B half-body steps
BODY = 2 * HB             # 32 steps per loop iteration
PAD = 2 * BODY            # xg stage slack read by the tail prefetches
STAGGERED = True
USE_IDMM = True           # accumulate xg into PSUM via identity matmuls

AF = mybir.ActivationFunctionType


def _build_gru(tc: tile.TileContext, aps: dict):
    nc = tc.nc
    x = aps["x"]                  # (T, BC, D)
    h0 = aps["initial_state"]     # (BC, H)
    W_ih = aps["W_ih"]            # (G3, D)
    W_hh = aps["W_hh"]            # (G3, H)
    b_ = aps["b"]                 # (G3,)
    b_n = aps["b_n"]              # (H,)
    y = aps["y"]                  # (T, BC, H)
    xg_stage = aps["xg_stage"]    # (GC, 128, (T+PAD)*BC) fp16

    xg_r = xg_stage.rearrange("c p tb -> p c tb")
    y_r = y.rearrange("t b (k p) -> p k (t b)", p=128)
    h0_r = h0.rearrange("b (k p) -> p k b", p=128)

    with ExitStack() as octx:
        singles = octx.enter_context(tc.tile_pool(name="singles", bufs=1))

        # fp32 weight staging, cast to fp16 working copies
        Wih32 = singles.tile([128, DC, G3], F32)
        Wih_r = W_ih.rearrange("g (k p) -> p k g", p=128)
        for k in range(DC):
            nc.sync.dma_start(Wih32[:, k, :], Wih_r[:, k, :])
        Whh32 = singles.tile([128, KC, G3], F32)
        Whh_r = W_hh.rearrange("g (k p) -> p k g", p=128)
        for k in range(KC):
            nc.sync.dma_start(Whh32[:, k, :], Whh_r[:, k, :])
        b32 = singles.tile([1, G3], F32)
        nc.sync.dma_start(b32, b_.rearrange("(o g) -> o g", o=1))
        bn32 = singles.tile([1, H], F32)
        nc.sync.dma_start(bn32, b_n.rearrange("(o g) -> o g", o=1))

        Wih16 = singles.tile([128, DC, G3], F16)
        nc.vector.tensor_copy(Wih16, Wih32)
        Whh16 = singles.tile([128, KC, G3], F16)
        nc.vector.tensor_copy(Whh16, Whh32)
        b16 = singles.tile([1, G3], F16)
        nc.vector.tensor_copy(b16, b32)
        bn16 = singles.tile([1, H], F16)
        nc.vector.tensor_copy(bn16, bn32)
        ones_bc = singles.tile([1, BC], F16)
        nc.vector.memset(ones_bc, 1.0)
        onesA = singles.tile([1, TBA * BC], F16)
        nc.vector.memset(onesA, 1.0)
        ident = singles.tile([128, 128], F16)
        make_identity(nc, ident)

        # ---------------- Phase A: xg = x @ W_ih.T + b (fp16) -----------
        with ExitStack() as actx:
            a_in = actx.enter_context(tc.tile_pool(name="a_in", bufs=2))
            a_xt = actx.enter_context(tc.tile_pool(name="a_xt", bufs=2))
            a_out = actx.enter_context(tc.tile_pool(name="a_out", bufs=2))
            a_ps = actx.enter_context(
                tc.tile_pool(name="a_ps", bufs=3, space="PSUM"))

            NTOK = TBA * BC  # 512 tokens per block
            for blk in range(NBA):
                xin = a_in.tile([128, 4, DC, 128], F32)
                for g in range(4):
                    t0 = blk * TBA + g * (TBA // 4)
                    nc.sync.dma_start(
                        xin[:, g],
                        x[t0:t0 + TBA // 4].rearrange(
                            "t b (k d) -> (t b) k d", d=128))
                xc16 = a_in.tile([128, 4, DC, 128], F16, tag="xc16")
                nc.vector.tensor_copy(xc16, xin)
                xT = a_xt.tile([128, DC, NTOK], F16)
                for g in range(4):
                    for kd in range(DC):
                        nc.sync.dma_start_transpose(
                            xT[:, kd, ts(g, 128)], xc16[:, g, kd])
                xga = a_out.tile([128, GC, NTOK], F16)
                for c in range(GC):
                    ps = a_ps.tile([128, NTOK], F32)
                    nc.tensor.matmul(ps, lhsT=b16[0:1, ts(c, 128)],
                                     rhs=onesA[0:1, :], start=True, stop=False)
                    for kd in range(DC):
                        nc.tensor.matmul(ps, lhsT=Wih16[:, kd, ts(c, 128)],
                                         rhs=xT[:, kd, :],
                                         start=False, stop=(kd == DC - 1))
                    nc.vector.tensor_copy(xga[:, c, :], ps)
                nc.sync.dma_start(xg_r[:, :, ds(blk * NTOK, NTOK)], xga)

        # Phase A writes xg_stage (raw DRAM tensor, not a pool tile) and
        # phase B reads it; force ordering across the DMA queues.
        tc.strict_bb_all_engine_barrier()

        # ---------------- Phase B: recurrence ----------------
        with ExitStack() as bctx:
            stat = bctx.enter_context(tc.tile_pool(name="stat", bufs=1))
            ping = bctx.enter_context(tc.tile_pool(name="ping", bufs=1))
            ps_rz = bctx.enter_context(
                tc.tile_pool(name="ps_rz", bufs=2, space="PSUM"))
            ps_c2 = bctx.enter_context(
                tc.tile_pool(name="ps_c2", bufs=2, space="PSUM"))
            sm = bctx.enter_context(tc.tile_pool(name="sm", bufs=3))

            # persistent state
            h16 = stat.tile([128, KC, BC], F16)
            h0_32 = stat.tile([128, KC, BC], F32)
            for k in range(KC):
                nc.sync.dma_start(h0_32[:, k, :], h0_r[:, k, :])
            nc.vector.tensor_copy(h16, h0_32)

            # ping-pong xg input and y staging buffers
            xg_sb = [ping.tile([128, GC, HB * BC], F16, name=f"xg{i}",
                               tag=f"xg{i}") for i in range(2)]
            hh = [ping.tile([128, KC, HB * BC], F16, name=f"hh{i}",
                            tag=f"hh{i}") for i in range(2)]
            yy = [ping.tile([128, KC, HB * BC], F32, name=f"yy{i}",
                            tag=f"yy{i}") for i in range(2)]

            # prologue loads
            nc.sync.dma_start(xg_sb[0], xg_r[:, :, 0:HB * BC])
            nc.sync.dma_start(xg_sb[1], xg_r[:, :, HB * BC:BODY * BC])

            def step(xg_half, hh_half, u):
                """One GRU step; all APs static. u is the python-static
                within-half step index."""
                xs = slice(u * BC, (u + 1) * BC)
                rz_ps = ps_rz.tile([128, 4, BC], F32)
                if USE_IDMM:
                    # deposit xg_rz first (no h dependency), weight matmuls
                    # accumulate on top. start=True clears the whole PSUM
                    # bank, so only the first matmul in the bank may set it.
                    for c in range(4):
                        nc.tensor.matmul(
                            rz_ps[:, c, :],
                            lhsT=ident,
                            rhs=xg_half[:, c, xs],
                            start=(c == 0), stop=False,
                            skip_group_check=True)
                for c in range(4):
                    for k in range(KC):
                        nc.tensor.matmul(
                            rz_ps[:, c, :],
                            lhsT=Whh16[:, k, ts(c, 128)],
                            rhs=h16[:, k, :],
                            start=(not USE_IDMM and k == 0),
                            stop=(k == KC - 1),
                            skip_group_check=True)
                if not USE_IDMM:
                    nc.vector.tensor_add(rz_ps, rz_ps, xg_half[:, 0:4, xs])
                c2_ps = ps_c2.tile([128, 2, BC], F32)
                for cc in range(2):
                    c = 4 + cc
                    nc.tensor.matmul(
                        c2_ps[:, cc, :],
                        lhsT=bn16[0:1, ts(cc, 128)],
                        rhs=ones_bc[0:1, :], start=True, stop=False)
                    for k in range(KC):
                        nc.tensor.matmul(
                            c2_ps[:, cc, :],
                            lhsT=Whh16[:, k, ts(c, 128)],
                            rhs=h16[:, k, :],
                            start=False, stop=(k == KC - 1))
                rz16 = sm.tile([128, 4, BC], F16, tag="rz")
                nc.scalar.activation(rz16, rz_ps, AF.Sigmoid)
                t1 = sm.tile([128, 2, BC], F16, tag="t1")
                nc.vector.tensor_mul(t1, rz16[:, 0:2, :], c2_ps)
                nc.vector.tensor_add(c2_ps, t1, xg_half[:, 4:6, xs])
                n16 = sm.tile([128, 2, BC], F16, tag="n")
                nc.scalar.activation(n16, c2_ps, AF.Tanh)
                u16 = sm.tile([128, 2, BC], F16, tag="u")
                nc.vector.tensor_sub(u16, h16, n16)
                nc.vector.tensor_mul(u16, rz16[:, 2:4, :], u16)
                nc.vector.tensor_add(h16, n16, u16)
                # stage output (off the critical chain)
                nc.gpsimd.tensor_copy(hh_half[:, :, xs], h16)

            def half(iv, i):
                for u in range(HB):
                    step(xg_sb[i], hh[i], u)
                nc.vector.tensor_copy(yy[i], hh[i])
                for k in range(KC):
                    nc.sync.dma_start(
                        y_r[:, k, ds((iv + i * HB) * BC, HB * BC)],
                        yy[i][:, k, :])
                # refill this half's xg for iteration iv + BODY
                nc.sync.dma_start(
                    xg_sb[i],
                    xg_r[:, :, ds((iv + BODY + i * HB) * BC, HB * BC)])

            with tc.For_i(0, T, BODY, staggered_reset=STAGGERED,
                          hint_engines=(mybir.EngineType.PE,)) as iv:
                half(iv, 0)
                half(iv, 1)


_BUILT = None


def _build():
    global _BUILT
    if _BUILT is not None:
        return _BUILT
    nc = bacc.Bacc("TRN2", target_bir_lowering=False, debug=False,
                   num_devices=NCORES)
    aps = {}
    aps["x"] = nc.dram_tensor("x", (T, BC, D), F32, kind="ExternalInput").ap()
    aps["initial_state"] = nc.dram_tensor(
        "initial_state", (BC, H), F32, kind="ExternalInput").ap()
    aps["W_ih"] = nc.dram_tensor("W_ih", (G3, D), F32,
                                 kind="ExternalInput").ap()
    aps["W_hh"] = nc.dram_tensor("W_hh", (G3, H), F32,
                                 kind="ExternalInput").ap()
    aps["b"] = nc.dram_tensor("b", (G3,), F32, kind="ExternalInput").ap()
    aps["b_n"] = nc.dram_tensor("b_n", (H,), F32, kind="ExternalInput").ap()
    aps["y"] = nc.dram_tensor("y", (T, BC, H), F32,
                              kind="ExternalOutput").ap()
    aps["xg_stage"] = nc.dram_tensor(
        "xg_stage", (GC, 128, (T + PAD) * BC), F16, kind="Internal").ap()
    with tile.TileContext(nc) as tc:
        _build_gru(tc, aps)
    nc.compile()
    _BUILT = nc
    return nc


def run(inputs: dict, trace: bool = False):
    nc = _build()
    in_maps = []
    for i in range(NCORES):
        sl = slice(i * BC, (i + 1) * BC)
        in_maps.append({
            "x": np.ascontiguousarray(
                np.asarray(inputs["x"], dtype=np.float32)[:, sl, :]),
            "initial_state": np.ascontiguousarray(
                np.asarray(inputs["initial_state"], dtype=np.float32)[sl]),
            "W_ih": np.ascontiguousarray(
                np.asarray(inputs["W_ih"], dtype=np.float32)),
            "W_hh": np.ascontiguousarray(
                np.asarray(inputs["W_hh"], dtype=np.float32)),
            "b": np.ascontiguousarray(
                np.asarray(inputs["b"], dtype=np.float32)),
            "b_n": np.ascontiguousarray(
                np.asarray(inputs["b_n"], dtype=np.float32)),
        })
    res = bass_utils.run_bass_kernel_spmd(
        nc, in_maps, core_ids=list(range(NCORES)), trace=trace)
    outs = res.results
    out = np.concatenate([outs[i]["y"] for i in range(NCORES)], axis=1)
    return out.astype(np.float32), res


def kernel(**inputs) -> np.ndarray:
    out, _ = run(inputs, trace=False)
    return out
